# revision 10
# baseline (speedup 1.0000x reference)
"""Trainium2 Bass kernel for nn_CubeSimulator.

Reference computation: a 128^3 spatial grid is rotated (Rz(sky_rot) then
Rx(inclination)), a rotation-curve velocity field and an exponential-disk
intensity field are evaluated, an 80-channel Gaussian KDE over the
line-of-sight velocity reduces the third grid axis, and the [80,128,128]
cube is avg-pooled (5,4,4) to [16,32,32].

Kernel strategy
---------------
* Exact point-reflection symmetry (as the previous version): only the i < 64
  half is computed on-device; the host mirrors the pooled output.
* Sharding: 8 computed sky-plane rows per core over 8 NeuronCores.
* On-device layout: partitions = k (the reduced axis), free = (i_local=8) x
  (j=128) = 1024.
* KDE via geometric channel recurrence instead of one Exp per channel:
  consecutive channels satisfy  w_{v+1} = exp(dz*2*vz/sig^2) * w_v * const_v.
  Per velocity-POOL group (5 channels) one anchor weight is computed exactly
  (ACT Exp of arg_a = zv2_a*vz + b, bias c_a), and the other 4 channels are
  bf16 tensor_tensor multiplies by t = exp(dz2*vz) (up) or 1/t (down) --
  2x-mode DVE/Pool ops, no ACT.  The per-channel constant exp(c_v - c_a) is
  folded into the matmul STATIONARY (Ldweights is free): only PSUM rows 0
  and 64 are ever read, so a [128,128] scale tile provides per-channel
  stationary columns via a sliding 64-wide window.
* The k-reduction and velocity pooling stay in PE/PSUM: 2 matmuls per
  channel accumulating 5 channels per PSUM bank.
* Field algebra is host-expanded: r2d2 = (A+nsz)^2 + rx2 and z2 = (B+ciz)^2
  arrive as  h0 + h1*nsz + nsz^2  forms -> one Pool/DVE stt + one
  tensor_scalar each.  Reciprocal via the 1-op approx (18 bits).
* All input-dependent scalars enter as DRAM tensors, so the compiled
  program is input-independent and cached.
"""

import sys

for _p in ("/opt/trn_rl_repo",):
    if _p not in sys.path:
        sys.path.insert(0, _p)

import numpy as np
import ml_dtypes

# ---------------- problem constants (compile-time, model-intrinsic) --------
IMAGE_RES = 128          # internal spatial resolution
VEL_RES = 80             # internal velocity channels
VEL_UP = 5
IMG_UP = 4
N_CORES = 8
HALF_I = IMAGE_RES // 2            # 64 computed rows
ROWS_PER_CORE = HALF_I // N_CORES  # 8
FREE = ROWS_PER_CORE * IMAGE_RES   # 1024 free elements per partition
HALF = FREE // 2
GROUPS = VEL_RES // VEL_UP         # 16 velocity-pool groups
OFFS = (1, 2, -1, -2)              # chain offsets from each group anchor
CUBE_FOV = 1000.0
M_TO_PC = 1.0 / 3.086e16
V_MAX_PC = np.float32(200000.0 * M_TO_PC)
R_C = np.float32(0.1 * CUBE_FOV)
R_D = np.float32(0.3 * CUBE_FOV)
H_Z = np.float32(0.05 * CUBE_FOV)
VEL_MIN = -300000.0
VEL_MAX = 300000.0

_INV_RD2 = 1.0 / (float(R_D) * float(R_D))  # Sqrt scale -> r2d/R_D
_EPS_RX2 = np.float32(1e-25)   # host-folded guard for the reciprocal

# scalar-column layout inside the packed small input `sm`
_C_NSZ = 0      # -si * z_k
_C_CIZ = 1      # ci * z_k
_C_NSZ2 = 2     # (si * z_k)^2
_C_CZ2 = 3      # (ci * z_k)^2
_C_ZV2 = 4                     # per-anchor 2*z_a/sig^2 (16)
_C_CA = _C_ZV2 + GROUPS        # per-anchor -z_a^2/sig^2 (16)
_C_DZ2 = _C_CA + GROUPS        # 36: 2*dz/sig^2
_C_NDZ2 = _C_DZ2 + 1           # 37
_C_RC2 = _C_NDZ2 + 1           # 38
_C_NSIG = _C_RC2 + 1           # 39: -1/sig^2
_C_NEGH = _C_NSIG + 1          # 40: -1/(2 H_Z^2)
SM_COLS = _C_NEGH + 1          # 41

# pk column blocks (each FREE wide): h0p, h1c, hB2, h2c, pc
_PK_H0P = 0
_PK_H1C = 1
_PK_HB2 = 2
_PK_H2C = 3
_PK_PC = 4
PK_COLS = 5 * FREE

_SPLIT_GROUPS = 2   # first groups emitted per-half to bridge startup

_CACHE = {}


def _build_program():
    from concourse import bacc, mybir, tile

    f32 = mybir.dt.float32
    bf16 = mybir.dt.bfloat16
    AF = mybir.ActivationFunctionType
    OP = mybir.AluOpType

    nc = bacc.Bacc(None)

    pk_d = nc.dram_tensor("pk", [128, PK_COLS], f32, kind="ExternalInput")
    sm_d = nc.dram_tensor("sm", [128, SM_COLS], f32, kind="ExternalInput")
    sta_d = nc.dram_tensor("sta", [128, 128], bf16, kind="ExternalInput")
    out_d = nc.dram_tensor("out", [16, 1024], f32, kind="ExternalOutput")

    with tile.TileContext(nc) as tc:
        with (
            tc.tile_pool(name="inp", bufs=1) as inp,
            tc.tile_pool(name="fld", bufs=1) as fld,
            tc.tile_pool(name="argp", bufs=4) as argp,
            tc.tile_pool(name="wp", bufs=14) as wp,
            tc.tile_pool(name="psum", bufs=5, space="PSUM") as psum,
            tc.tile_pool(name="obp", bufs=4) as obp,
        ):
            pk = inp.tile([128, PK_COLS], f32)
            sm = inp.tile([128, SM_COLS], f32)
            sta = inp.tile([128, 128], bf16)
            V = nc.vector
            G = nc.gpsimd
            A = nc.scalar

            def pcol(blk, h):
                return slice(blk * FREE + h * HALF, blk * FREE + (h + 1) * HALF)

            # DMA order = consumption order; h0 chunks on the sync queue,
            # h1 chunks on the gpsimd queue so both field chains start early.
            nc.sync.dma_start(sm[:], sm_d[:])
            nc.gpsimd.dma_start(sta[:], sta_d[:])
            for blk in (_PK_H1C, _PK_H0P, _PK_H2C, _PK_HB2, _PK_PC):
                nc.sync.dma_start(pk[:, pcol(blk, 0)], pk_d[:, pcol(blk, 0)])
                nc.gpsimd.dma_start(pk[:, pcol(blk, 1)], pk_d[:, pcol(blk, 1)])

            def col(i):
                return sm[:, i:i + 1]

            # ---------------- field ----------------
            ra = fld.tile([128, FREE], f32)
            z2a = fld.tile([128, FREE], f32)
            r2d2 = fld.tile([128, FREE], f32)
            z2 = fld.tile([128, FREE], f32)
            q = fld.tile([128, FREE], f32)
            den = fld.tile([128, FREE], f32)
            rec = fld.tile([128, FREE], f32)
            u = fld.tile([128, FREE], f32)
            su = fld.tile([128, FREE], f32)
            vzt = fld.tile([128, FREE], f32)
            nvs = fld.tile([128, FREE], f32)
            nvt = fld.tile([128, FREE], f32)
            qc = fld.tile([128, FREE], f32)
            gt1 = fld.tile([128, FREE], f32)
            gt2 = fld.tile([128, FREE], f32)
            t2 = fld.tile([128, FREE], f32)
            slq = fld.tile([128, FREE], f32)
            bb = fld.tile([128, FREE], f32)
            tb = fld.tile([128, FREE], bf16)
            tnv = fld.tile([128, FREE], bf16)

            def sl(h):
                return slice(h * HALF, (h + 1) * HALF)

            # spine engine per half: h0 -> DVE, h1 -> Pool; side swapped.
            # Pool cannot run scalar_tensor_tensor (neff codegen rejects it);
            # decompose into tensor_scalar + tensor_tensor there.
            def stt_ma(E, out, in0, c, in1, tmp):
                """out = in0*col(c) + in1 on engine E (stt or 2-op)."""
                if E is V:
                    E.scalar_tensor_tensor(out, in0, col(c), in1,
                                           op0=OP.mult, op1=OP.add)
                else:
                    E.tensor_scalar_mul(tmp, in0, col(c))
                    E.tensor_add(out, tmp, in1)

            for h, (S, D) in ((0, (V, G)), (1, (G, V))):
                s = sl(h)
                stt_ma(S, ra[:, s], pk[:, pcol(_PK_H1C, h)], _C_NSZ,
                       pk[:, pcol(_PK_H0P, h)], gt1[:, s])
                S.tensor_scalar_add(r2d2[:, s], ra[:, s], col(_C_NSZ2))
                stt_ma(D, z2a[:, s], pk[:, pcol(_PK_H2C, h)], _C_CIZ,
                       pk[:, pcol(_PK_HB2, h)], gt2[:, s])
                D.tensor_scalar_add(z2[:, s], z2a[:, s], col(_C_CZ2))
                S.tensor_add(q[:, s], r2d2[:, s], z2[:, s])
                # den = (q + R_C^2) * r2d2; Pool stt only supports
                # (mult, add), so decompose on the h1 spine
                if S is V:
                    S.scalar_tensor_tensor(den[:, s], q[:, s], col(_C_RC2),
                                           r2d2[:, s], op0=OP.add, op1=OP.mult)
                else:
                    G.tensor_scalar_add(qc[:, s], q[:, s], col(_C_RC2))
                    G.tensor_mul(den[:, s], qc[:, s], r2d2[:, s])
                V.reciprocal_approx_fast(rec[:, s], den[:, s])
                S.tensor_mul(u[:, s], q[:, s], rec[:, s])
                A.activation(su[:, s], u[:, s], AF.Sqrt)
                S.tensor_mul(vzt[:, s], su[:, s], pk[:, pcol(_PK_PC, h)])
                A.activation(slq[:, s], r2d2[:, s], AF.Sqrt, scale=_INV_RD2)
                # nvs = -(vz/sig)^2 ; h0 via DVE stt, h1 via Pool 2-op
                if h == 0:
                    V.scalar_tensor_tensor(nvs[:, s], vzt[:, s], col(_C_NSIG),
                                           vzt[:, s], op0=OP.mult, op1=OP.mult)
                else:
                    G.tensor_scalar_mul(nvt[:, s], vzt[:, s], col(_C_NSIG))
                    G.tensor_mul(nvs[:, s], nvt[:, s], vzt[:, s])
                stt_ma(D, t2[:, s], z2[:, s], _C_NEGH, nvs[:, s], gt1[:, s])
                D.tensor_sub(bb[:, s], t2[:, s], slq[:, s])
                # chain multipliers t = exp(dz2*vz), tinv = exp(-dz2*vz)
                A.activation(tb[:, s], vzt[:, s], AF.Exp, scale=col(_C_DZ2))
                A.activation(tnv[:, s], vzt[:, s], AF.Exp, scale=col(_C_NDZ2))

            # ---------------- KDE: 16 groups, anchor + chain ----------------
            args = {}
            wtiles = {}

            def emit_arg(g, hs):
                at = args.get(g)
                if at is None:
                    at = argp.tile([128, FREE], f32, tag="arg", name=f"arg{g}")
                    args[g] = at
                V.scalar_tensor_tensor(at[:, hs], vzt[:, hs], col(_C_ZV2 + g),
                                       bb[:, hs], op0=OP.mult, op1=OP.add)

            def emit_anchor(g, hs):
                wt = wtiles.get((g, 0))
                if wt is None:
                    wt = wp.tile([128, FREE], bf16, tag="w", name=f"w{g}_0")
                    wtiles[(g, 0)] = wt
                A.activation(wt[:, hs], args[g][:, hs], AF.Exp,
                             bias=col(_C_CA + g))

            def emit_step(g, off, hs, E):
                src = wtiles[(g, 0 if abs(off) == 1 else (1 if off > 0 else -1))]
                wt = wtiles.get((g, off))
                if wt is None:
                    wt = wp.tile([128, FREE], bf16, tag="w", name=f"w{g}_{off}")
                    wtiles[(g, off)] = wt
                mult = tb if off > 0 else tnv
                E.tensor_mul(wt[:, hs], mult[:, hs], src[:, hs])

            psum_tiles = {}

            def emit_mms(g, off, start, stop, h=None):
                pt = psum_tiles.get(g)
                if pt is None:
                    pt = psum.tile([128, HALF], f32, tag="acc", name=f"acc{g}")
                    psum_tiles[g] = pt
                if off == 0:
                    st_ap = sta[:, 64:128]
                else:
                    oi = OFFS.index(off)
                    ix = 4 * g + oi
                    st_ap = sta[:, ix:ix + 64]
                wt = wtiles[(g, off)]
                halves = (0, 1) if h is None else (h,)
                for hh in halves:
                    nc.tensor.matmul(
                        pt[64 * hh:64 * hh + 64, :], st_ap,
                        wt[:, sl(hh)], start=start, stop=stop,
                        skip_group_check=True)

            def emit_copy_dma(g):
                pt = psum_tiles.pop(g)
                ot = obp.tile([65, HALF], f32, tag="ob", name=f"ot{g}")
                A.activation(ot[:, :], pt[0:65, :], AF.Copy)
                nc.sync.dma_start(
                    out_d[g, :].rearrange("(q n) -> q n", q=2),
                    ot[0:65:64, :])

            # --- split (per-half) groups bridge the field->KDE transition
            def step_engines(g):
                # DVE carries the arg stt; balance steps toward Pool
                if g % 2 == 0:
                    return {1: V, 2: V, -1: G, -2: G}
                return {1: V, 2: G, -1: G, -2: G}

            for g in range(_SPLIT_GROUPS):
                se = step_engines(g)
                for h in (0, 1):
                    hs = sl(h)
                    emit_arg(g, hs)
                    emit_anchor(g, hs)
                    emit_mms(g, 0, start=True, stop=False, h=h)
                    emit_step(g, 1, hs, se[1])
                    emit_mms(g, 1, start=False, stop=False, h=h)
                    emit_step(g, 2, hs, se[2])
                    emit_mms(g, 2, start=False, stop=False, h=h)
                    emit_step(g, -1, hs, se[-1])
                    emit_mms(g, -1, start=False, stop=False, h=h)
                    emit_step(g, -2, hs, se[-2])
                    emit_mms(g, -2, start=False, stop=True, h=h)

            # --- steady-state groups, full-width, software-pipelined
            full = slice(0, FREE)
            for g in range(_SPLIT_GROUPS, GROUPS):
                emit_arg(g, full)
            # (args all emitted up-front on Pool; they only need vzt/bb and
            #  stay ahead of the chain steps in Pool's queue)
            for g in range(_SPLIT_GROUPS, GROUPS):
                se = step_engines(g)
                emit_anchor(g, full)
                emit_mms(g, 0, start=True, stop=False)
                emit_step(g, 1, full, se[1])
                emit_mms(g, 1, start=False, stop=False)
                emit_step(g, 2, full, se[2])
                emit_mms(g, 2, start=False, stop=False)
                emit_step(g, -1, full, se[-1])
                emit_mms(g, -1, start=False, stop=False)
                emit_step(g, -2, full, se[-2])
                emit_mms(g, -2, start=False, stop=True)
                if g - 1 >= 0 and (g - 1) in psum_tiles:
                    emit_copy_dma(g - 1)
            emit_copy_dma(GROUPS - 1)
            # drain any split-group psums not yet copied (g- ordering covers
            # all but ensure none left)
            for g in sorted(psum_tiles):
                emit_copy_dma(g)

    nc.finalize()
    return nc


def _host_inputs(inclination, sky_rot, line_broadening):
    f32 = np.float32
    f64 = np.float64
    inc = f32(inclination)
    rot = f32(sky_rot)
    lb = f32(line_broadening)
    ci, si = f32(np.cos(inc)), f32(np.sin(inc))
    cr, sr = f32(np.cos(rot)), f32(np.sin(rot))
    sig_sq = f32(lb * lb)
    s2 = f64(sig_sq)

    lin = np.linspace(-CUBE_FOV, CUBE_FOV, IMAGE_RES, dtype=f32)
    z_labels = np.linspace(f32(VEL_MIN * M_TO_PC), f32(VEL_MAX * M_TO_PC),
                           VEL_RES, dtype=f32)
    zs = z_labels.astype(f64)
    dz = (zs[-1] - zs[0]) / (VEL_RES - 1)
    cvals = -(zs * zs) / s2                      # c_v (f64), range ~[-223, 0]

    sm = np.zeros((128, SM_COLS), dtype=f32)
    sm[:, _C_NSZ] = (-si * lin).astype(f32)
    sm[:, _C_CIZ] = (ci * lin).astype(f32)
    sm[:, _C_NSZ2] = (sm[:, _C_NSZ] ** 2).astype(f32)
    sm[:, _C_CZ2] = (sm[:, _C_CIZ] ** 2).astype(f32)
    for g in range(GROUPS):
        a = 5 * g + 2
        sm[:, _C_ZV2 + g] = f32(2.0 * zs[a] / s2)
        sm[:, _C_CA + g] = f32(cvals[a])
    sm[:, _C_DZ2] = f32(2.0 * dz / s2)
    sm[:, _C_NDZ2] = -sm[:, _C_DZ2]
    sm[:, _C_RC2] = f32(float(R_C) * float(R_C))
    sm[:, _C_NSIG] = f32(-1.0 / sig_sq)
    sm[:, _C_NEGH] = f32(-1.0 / (2.0 * float(H_Z) * float(H_Z)))

    sta = np.ones((128, 128), dtype=np.float32)
    for g in range(GROUPS):
        a = 5 * g + 2
        for oi, off in enumerate(OFFS):
            sta[:, 4 * g + oi] = f32(np.exp(cvals[a + off] - cvals[a]))
    sta = sta.astype(ml_dtypes.bfloat16)

    in_maps = []
    for c in range(N_CORES):
        x = lin[8 * c: 8 * c + 8][:, None]                 # [8,1]
        y = lin[None, :]                                   # [1,128]
        y1 = (sr * x + cr * y).astype(f32)
        Acol = (ci * y1).astype(f32)
        Bcol = (si * y1).astype(f32)
        rot_x = (cr * x - sr * y).astype(f32)
        h0p = (Acol * Acol + rot_x * rot_x + _EPS_RX2).astype(f32).reshape(-1)
        h1c = (2.0 * Acol).astype(f32).reshape(-1)
        hB2 = (Bcol * Bcol).astype(f32).reshape(-1)
        h2c = (2.0 * Bcol).astype(f32).reshape(-1)
        pc = (-si * V_MAX_PC * rot_x).astype(f32).reshape(-1)
        pkrow = np.concatenate([h0p, h1c, hB2, h2c, pc]).astype(f32)
        pk = np.ascontiguousarray(np.broadcast_to(pkrow, (128, PK_COLS)))
        in_maps.append({"pk": pk, "sm": sm, "sta": sta})
    return in_maps


def _run(in_maps, trace=False, **kwargs):
    from concourse.bass_utils import run_bass_kernel_spmd
    if "nc" not in _CACHE:
        _CACHE["nc"] = _build_program()
    return run_bass_kernel_spmd(_CACHE["nc"], in_maps,
                                list(range(N_CORES)), trace=trace, **kwargs)


def _assemble(results, line_broadening):
    f32 = np.float32
    lb = f32(line_broadening)
    sig_sq = f32(lb * lb)
    pref = f32(1.0 / np.sqrt(2.0 * np.pi * sig_sq))
    scale = f32(pref / f32(VEL_UP * IMG_UP * IMG_UP))
    parts = []
    for r in results:
        cube = np.asarray(r["out"]).reshape(16, 2, 4, 32, 4)  # vo,io,di,jo,dj
        pooled = cube.sum(axis=(2, 4), dtype=np.float32) * scale  # [16,2,32]
        parts.append(pooled.astype(f32))
    half = np.concatenate(parts, axis=1)
    full = np.empty((16, 32, 32), dtype=np.float32)
    full[:, :16, :] = half
    full[:, 16:, :] = half[::-1, ::-1, ::-1]
    return full


def kernel(inclination, sky_rot, line_broadening):
    in_maps = _host_inputs(inclination, sky_rot, line_broadening)
    res = _run(in_maps)
    return _assemble(res.results, line_broadening)
